# revision 1
# baseline (speedup 1.0000x reference)
"""Trainium2 Bass kernel for nn_Apply_Mask (topk_masking). Final (v14).

Per (batch, channel) slice of shape 32x32: find the argmax location, build
a clipped 5x5 box around it, S = 1 - box, lam = 1024/sum(S), and
out = (T != 0) ? x * S * lam : x.

Sharding: data-parallel over the 32768 b*c slices; core i takes slices
[4096*i, 4096*(i+1)). Per-core layout: partition p holds slices
[32p, 32p+32); tile t = slice 32p+t at free offset t*1024.

Math: with sel = (T != 0), a = sel ? lam : 1 and the binary box mask
q2 = (row_in * sel) (x) col_in, the output is
    out = a * u,  u = (q2 == 0) ? x : 0.
The select uses a scalar immediate so it batches 4 tiles per
scalar_tensor_tensor; the per-tile a and the f32->bf16 downcast ride the
ScalarE activation copy for free.

Engine split: DVE does the exact f32 argmax (per-tile max8 + 4-tile
batched find_index8 -- windows kept at 4 tiles to avoid value-collision
mis-argmax), iota compares, per-slice scalar math, and the batched
select; GpSimd does mask arithmetic + outer products (its ISA only
allows arithmetic TT ops); ScalarE does out = bf16(a*u); DMA moves
f32 in (4 x 4 MiB chunks), bf16 out (8 MiB). Both outer products of a
group are emitted before its selects so each select reads a one-batch
stale q (avoids the ~1.1us Pool->DVE handoff stall). Tiles process in
4 groups so group g's mask/apply overlaps group g+1's argmax.

HW-verified: rel err 1.66e-3 vs reference (pure bf16 output rounding;
compute is bit-exact f32). ~200-242us/core measured across machine-load
windows; memory roofline ~93us.
"""
import sys

for _p in ("/opt/trn_rl_repo",):
    if _p not in sys.path:
        sys.path.insert(0, _p)

import numpy as np

import concourse.bass as bass
import concourse.tile as tile
from concourse import bacc, mybir
from concourse.bass_utils import run_bass_kernel_spmd

P = 128
NT = 32
H = W = 32
HW = H * W
N_CORES = 8
SLICES_PER_CORE = P * NT

OUT_BF16 = True
KQ = 4
NGROUP = 4

f32 = mybir.dt.float32
bf16 = mybir.dt.bfloat16
u16 = mybir.dt.uint16
Alu = mybir.AluOpType
Act = mybir.ActivationFunctionType

_cached = {}


def _build(half: int):
    odt = bf16 if OUT_BF16 else f32
    GT = NT // NGROUP
    NB = NT // KQ

    nc = bacc.Bacc("TRN2", target_bir_lowering=False, debug=False,
                   num_devices=N_CORES)
    x_in = nc.dram_tensor("x", [P, NT * HW], f32, kind="ExternalInput").ap()
    sel_in = nc.dram_tensor("sel", [P, NT], f32, kind="ExternalInput").ap()
    io_in = nc.dram_tensor("io32", [P, 32], f32, kind="ExternalInput").ap()
    out_d = nc.dram_tensor("out", [P, NT * HW], odt, kind="ExternalOutput").ap()

    with tile.TileContext(nc) as tc:
        from contextlib import ExitStack
        with ExitStack() as ctx:
            xpool = ctx.enter_context(tc.tile_pool(name="xp", bufs=1))
            mid = ctx.enter_context(tc.tile_pool(name="mid", bufs=1))
            small = ctx.enter_context(tc.tile_pool(name="small", bufs=1))
            qpool = ctx.enter_context(tc.tile_pool(name="qp", bufs=2))
            opool = ctx.enter_context(tc.tile_pool(name="op", bufs=2))

            CH = 2 * KQ          # tiles per x chunk (= one group)
            xc = []
            for c_ in range(NT // CH):
                t_ = xpool.tile([P, CH * HW], f32, name=f"x{c_}", tag=f"x{c_}")
                nc.sync.dma_start(t_[:], x_in[:, c_ * CH * HW:(c_ + 1) * CH * HW])
                xc.append(t_)

            def x_tile(t):
                return xc[t // CH][:, (t % CH) * HW:(t % CH + 1) * HW]

            def x_batch(b):
                # KQ-tile slice of the owning chunk for batch b
                tb = b * KQ
                return xc[tb // CH][:, (tb % CH) * HW:(tb % CH + KQ) * HW]

            selp = small.tile([P, NT], f32)
            nc.sync.dma_start(selp[:], sel_in)
            io32 = small.tile([P, 32], f32)
            nc.sync.dma_start(io32[:], io_in)

            max8 = mid.tile([P, NT, 8], f32)
            idx8 = mid.tile([P, NB, 8], u16)
            col_in = mid.tile([P, NT, W], f32)
            col_gt = mid.tile([P, NT, W], f32)
            row_sl = mid.tile([P, NT, H], f32)
            row_gt = mid.tile([P, NT, H], f32)
            io_b = io32[:, None, :]

            def smalls(name, dt=f32):
                return [small.tile([P, GT], dt, name=f"{name}{g}", tag=f"{name}{g}")
                        for g in range(NGROUP)]

            idx_u = smalls("idxu", u16)
            mh_u = smalls("mhu", u16)
            mw_u = smalls("mwu", u16)
            mh = smalls("mh"); mw = smalls("mw")
            h1 = smalls("h1"); h2 = smalls("h2"); w1 = smalls("w1"); w2 = smalls("w2")
            rl = smalls("rl"); cl1 = smalls("cl1"); area = smalls("area")
            denom = smalls("denom"); recip = smalls("recip"); lam1 = smalls("lam1")
            a_t = smalls("a")

            for g in range(NGROUP):
                gl = g * GT
                gsl = slice(gl, gl + GT)
                b0 = gl // KQ
                nbg = GT // KQ

                for t in range(gl, gl + GT):
                    nc.vector.max(max8[:, t], x_tile(t))
                for b_ in range(b0, b0 + nbg):
                    inm = small.tile([P, 2 * KQ], f32, name=f"inm{b_}", tag="inm")
                    nc.vector.tensor_copy(
                        inm[:].rearrange("p (t k) -> p t k", t=KQ, k=2),
                        max8[:, b_ * KQ:(b_ + 1) * KQ, 0:2])
                    nc.vector.max_index(idx8[:, b_], inm[:], x_batch(b_))

                nc.vector.tensor_copy(
                    idx_u[g][:].rearrange("p (b j) -> p b j", b=nbg, j=KQ),
                    idx8[:, b0:b0 + nbg, 0:2 * KQ:2])
                nc.vector.tensor_scalar(mh_u[g][:], idx_u[g][:], 5, 31,
                                        Alu.logical_shift_right, Alu.bitwise_and)
                nc.vector.tensor_scalar(mw_u[g][:], idx_u[g][:], 31, None, Alu.bitwise_and)
                nc.vector.tensor_copy(mh[g][:], mh_u[g][:])
                nc.vector.tensor_copy(mw[g][:], mw_u[g][:])
                nc.vector.tensor_scalar(h1[g][:], mh[g][:], float(half), 0.0, Alu.subtract, Alu.max)
                nc.vector.tensor_scalar(h2[g][:], mh[g][:], float(half), float(H - 1), Alu.add, Alu.min)
                nc.vector.tensor_scalar(w1[g][:], mw[g][:], float(half), 0.0, Alu.subtract, Alu.max)
                nc.vector.tensor_scalar(w2[g][:], mw[g][:], float(half), float(W - 1), Alu.add, Alu.min)
                nc.vector.tensor_tensor(rl[g][:], h2[g][:], h1[g][:], Alu.subtract)
                nc.vector.tensor_tensor(cl1[g][:], w2[g][:], w1[g][:], Alu.subtract)
                nc.vector.tensor_scalar(cl1[g][:], cl1[g][:], 1.0, None, Alu.add)
                nc.vector.scalar_tensor_tensor(area[g][:], rl[g][:], 1.0, cl1[g][:], Alu.add, Alu.mult)
                nc.vector.tensor_scalar(denom[g][:], area[g][:], -1.0, float(HW), Alu.mult, Alu.add)
                nc.vector.reciprocal(recip[g][:], denom[g][:])
                nc.vector.tensor_scalar(lam1[g][:], recip[g][:], float(HW), -1.0, Alu.mult, Alu.add)
                nc.vector.scalar_tensor_tensor(a_t[g][:], lam1[g][:], 0.0, selp[:, gsl], Alu.add, Alu.mult)
                nc.vector.tensor_scalar(a_t[g][:], a_t[g][:], 1.0, None, Alu.add)

                iog = io_b.broadcast_to([P, GT, 32])
                nc.vector.tensor_tensor(col_in[:, gsl], iog, w1[g][:, :, None].broadcast_to([P, GT, W]), Alu.is_ge)
                nc.vector.tensor_tensor(col_gt[:, gsl], iog, w2[g][:, :, None].broadcast_to([P, GT, W]), Alu.is_gt)
                nc.gpsimd.tensor_tensor(col_in[:, gsl], col_in[:, gsl], col_gt[:, gsl], Alu.subtract)
                nc.vector.tensor_tensor(row_sl[:, gsl], iog, h1[g][:, :, None].broadcast_to([P, GT, H]), Alu.is_ge)
                nc.vector.tensor_tensor(row_gt[:, gsl], iog, h2[g][:, :, None].broadcast_to([P, GT, H]), Alu.is_gt)
                nc.gpsimd.tensor_tensor(row_sl[:, gsl], row_sl[:, gsl], row_gt[:, gsl], Alu.subtract)
                nc.gpsimd.tensor_tensor(row_sl[:, gsl], row_sl[:, gsl], selp[:, gsl, None].broadcast_to([P, GT, H]), Alu.mult)

                qs = {}
                for b_ in range(b0, b0 + nbg):
                    tb = b_ * KQ
                    q = qpool.tile([P, KQ, H, W], f32, name=f"q{b_}", tag="q")
                    nc.gpsimd.tensor_tensor(
                        q[:],
                        row_sl[:, tb:tb + KQ, :, None].broadcast_to([P, KQ, H, W]),
                        col_in[:, tb:tb + KQ, None, :].broadcast_to([P, KQ, H, W]),
                        Alu.mult,
                    )
                    qs[b_] = q
                for b_ in range(b0, b0 + nbg):
                    tb = b_ * KQ
                    q = qs[b_]
                    nc.vector.scalar_tensor_tensor(
                        q[:], q[:], 0.0,
                        x_batch(b_).rearrange("p (t h w) -> p t h w", t=KQ, h=H, w=W),
                        Alu.is_equal, Alu.mult,
                    )
                    o_c = opool.tile([P, KQ * HW], odt, name=f"o{b_}", tag="oc")
                    for j in range(KQ):
                        t = tb + j
                        nc.scalar.activation(
                            o_c[:, j * HW:(j + 1) * HW],
                            q[:, j].rearrange("p h w -> p (h w)"),
                            Act.Copy, bias=0.0, scale=a_t[g][:, t - gl, None],
                        )
                    nc.sync.dma_start(out_d[:, tb * HW:(tb + KQ) * HW], o_c[:])

    nc.compile()
    return nc


def _get_nc(half: int):
    if half not in _cached:
        _cached[half] = _build(half)
    return _cached[half]


def _shard_inputs(x, T):
    xf = np.ascontiguousarray(x, dtype=np.float32).reshape(-1, HW)
    sel = (np.asarray(T).reshape(-1) != 0).astype(np.float32)
    io32 = np.tile(np.arange(32, dtype=np.float32), (P, 1))
    in_maps = []
    for i in range(N_CORES):
        lo = i * SLICES_PER_CORE
        hi = lo + SLICES_PER_CORE
        in_maps.append({
            "x": np.ascontiguousarray(xf[lo:hi].reshape(P, NT * HW)),
            "sel": np.ascontiguousarray(sel[lo:hi].reshape(P, NT)),
            "io32": io32,
        })
    return in_maps


def run(inputs, trace=False, **kw):
    x = inputs["x"]
    T = inputs["T"]
    drop_block = int(np.asarray(inputs["drop_block"]))
    half = drop_block // 2
    b, c, h, w = x.shape
    assert (h, w) == (H, W) and b * c == N_CORES * SLICES_PER_CORE, \
        f"kernel hardcoded for (128,256,32,32); got {x.shape}"

    nc = _get_nc(half)
    in_maps = _shard_inputs(x, T)
    res = run_bass_kernel_spmd(nc, in_maps, core_ids=list(range(N_CORES)),
                               trace=trace, **kw)
    parts = [np.asarray(res.results[i]["out"]).astype(np.float32)
              .reshape(SLICES_PER_CORE, HW)
             for i in range(N_CORES)]
    out = np.concatenate(parts, axis=0).reshape(b, c, h, w)
    return out, res


def kernel(**inputs) -> np.ndarray:
    out, _ = run(inputs, trace=False)
    return out

